# revision 27
# baseline (speedup 1.0000x reference)
"""AblationPewLSTM Trainium2 kernel.

Model (B=1024 days, T=24, I=512, H=1024, W=64):
  lag inputs x_day/x_week/x_month = x_input shifted by 1/7/30 along batch
  e = sigmoid(x_weather @ w_e.T + b_e)                    (hoisted)
  p_per = x_day@w_d.T + x_week@w_w.T + x_month@w_m.T      (hoisted)
  p_i/f/o = x_input@w_{i,f,o}x.T + e@w_{i,f,o}e.T + b     (hoisted)
  p_g = x_input@w_gx.T + b_g                              (hoisted)
  per t: h_o = sig(p_per + 2 h w_t.T); i,f,o = sig(p_* + h_o W.T);
         g = tanh(p_g + h_o w_gh.T); c = f*c + i*g; h = o*tanh(c)
  returns (hs, cs) each [B, T, H] fp32.

Sharding: batch (day) dim across 8 cores, 128 days each; weights replicated.
Lag shifts resolved on host (slicing the full x_input), so no halo exchange.

Per-core kernel: all matmuls in bf16 (PE full rate; fp32 is 1/4 rate on trn2)
with fp32 PSUM accumulation and fp32 elementwise state.
Phase 1 computes the hoisted projections into a DRAM scratch, row-tile r == t
so phase 2 (the 24-step recurrence) pipelines behind it. Recurrent state is
kept as transposed [H,128] bf16 tiles (lhsT of the gate matmuls); h/h_o are
re-transposed each step on the PE.
"""

import numpy as np
import ml_dtypes

BF16 = ml_dtypes.bfloat16
B, T, I, H, W = 1024, 24, 512, 1024, 64
NCORES = 8
BC = B // NCORES          # 128 days per core
R = BC * T                # 3072 rows per core, r = t*BC + b
KI = I // 128             # 4 k-tiles for I-contraction
KH = H // 128             # 8 k-tiles for H-contraction
NCH = H // 512            # 2 psum n-chunks of 512

_CACHED = None


def _build():
    import concourse.bacc as bacc
    import concourse.tile as tile
    from concourse import mybir
    from concourse.masks import make_identity
    import concourse.bass as bass

    f32 = mybir.dt.float32
    bf16 = mybir.dt.bfloat16
    SIG = mybir.ActivationFunctionType.Sigmoid
    TANH = mybir.ActivationFunctionType.Tanh

    nc = bacc.Bacc("TRN2", target_bir_lowering=False)

    # ---- inputs ----
    def din(name, shape, dt=bf16):
        return nc.dram_tensor(name, shape, dt, kind="ExternalInput").ap()

    xt_in = din("xt_in", [I, R])          # x_input shard, [i, t*BC+b]
    xt_day = din("xt_day", [I, R])
    xt_week = din("xt_week", [I, R])
    xt_month = din("xt_month", [I, R])
    xwt = din("xwt", [W, R])              # weather transposed
    # x-side weights, transposed to [I, H]
    wxT = {g: din(f"w{g}xT", [I, H]) for g in ("d", "w", "m", "i", "f", "o", "g")}
    # e-side weights [H, H]
    weT = {g: din(f"w{g}eT", [H, H]) for g in ("i", "f", "o")}
    w_eT = din("w_eT", [W, H])            # w_e.T, stationary for eT pass
    b_e_t = din("b_e_t", [128, KH], f32)  # b_e.reshape(8,128).T
    bias = {g: din(f"b_{g}", [1, H], f32) for g in ("i", "f", "o", "g")}
    # recurrent weights [H, H] (w_t pre-scaled by 2)
    wrT = {g: din(f"wr_{g}T", [H, H]) for g in ("t2", "ih", "gh", "fo", "oh")}

    hs_out = nc.dram_tensor("hs", [BC, T, H], f32, kind="ExternalOutput").ap()
    cs_out = nc.dram_tensor("cs", [BC, T, H], f32, kind="ExternalOutput").ap()

    def bcast(ap_1xh):  # [1, H] dram -> partition-broadcast AP [128, H]
        return bass.AP(tensor=ap_1xh.tensor, offset=ap_1xh.offset,
                       ap=[[0, 128], ap_1xh.ap[-1]])

    with tile.TileContext(nc) as tc:
        with tc.tile_pool(name="persist", bufs=1) as persist, \
             tc.tile_pool(name="dram", bufs=1, space="DRAM") as dram:

            ident = persist.tile([128, 128], bf16)
            make_identity(nc, ident)

            # DRAM scratch: p_all[t, g, b, h], g order: i,f,o,g,per ; eT[k, h, r]
            p_all = dram.tile([T, 5, BC, H], bf16)
            eT_d = dram.tile([KH, 128, R], bf16)

            # ================= PHASE 1 =================
            # Recurrence weights prefetch at the very top: their pool lives
            # through the whole kernel, and phase 1's split passes keep the
            # combined footprint under the SBUF cap.
            p2w_cm = tc.tile_pool(name="p2w", bufs=1)
            p2w = p2w_cm.__enter__()
            wr_sb = {}

            with tc.tile_pool(name="p1A", bufs=1) as p1A, \
                 tc.tile_pool(name="p1x", bufs=4) as p1x, \
                 tc.tile_pool(name="p1s", bufs=6) as p1s, \
                 tc.tile_pool(name="p1ps", bufs=6, space="PSUM") as p1ps:

                # eT-pass inputs first so its MMs start ~10us in, ahead of
                # the weight DMA train.
                wet_sb = p1A.tile([W, H], bf16, tag="wet")
                nc.gpsimd.dma_start(wet_sb[:], w_eT)
                bet_sb = p1A.tile([128, KH], f32, tag="bet")
                nc.gpsimd.dma_start(bet_sb[:], b_e_t)
                xw_sb = p1A.tile([W, R], bf16, tag="xw")
                nc.sync.dma_start(xw_sb[:], xwt)

                wxA_sb = {}
                for g in ("d", "w", "m", "g"):
                    t_ = p1A.tile([128, KI, H], bf16, tag=f"wx{g}")
                    nc.gpsimd.dma_start(t_[:], wxT[g].rearrange("(k p) h -> p k h", p=128))
                    wxA_sb[g] = t_
                bg_sb = p1A.tile([128, H], f32, tag="bg")
                nc.gpsimd.dma_start(bg_sb[:], bcast(bias["g"]))

                # --- eT pass: eT[m*128+p, r] = sig(b_e + sum_w w_e.T[w, m] xw[w, r])
                for m in range(KH):
                    for ch in range(R // 512):
                        ps = p1ps.tile([128, 512], f32, tag="pps")
                        nc.tensor.matmul(ps[:], wet_sb[:, m * 128:(m + 1) * 128],
                                         xw_sb[:, ch * 512:(ch + 1) * 512],
                                         start=True, stop=True)
                        st = p1s.tile([128, 512], bf16, tag="pst")
                        nc.scalar.activation(st[:], ps[:], SIG,
                                             bias=bet_sb[:, m:m + 1], scale=1.0)
                        nc.sync.dma_start(eT_d[m, :, ch * 512:(ch + 1) * 512], st[:])

                # --- pass A: p_per (idx 4) and p_g (idx 3) per row-tile ---
                xt_lag = {"d": xt_day, "w": xt_week, "m": xt_month, "x": xt_in}
                for r in range(T):
                    if r == 6:
                        # recurrence weights trickle in behind the pass-A
                        # streams; needed only at ~650us
                        for g, d in wrT.items():
                            t_ = p2w.tile([128, KH, H], bf16, tag=f"wr{g}")
                            nc.gpsimd.dma_start(
                                t_[:], d.rearrange("(k p) h -> p k h", p=128))
                            wr_sb[g] = t_
                    cols = slice(r * BC, (r + 1) * BC)
                    xs = {}
                    for key, d in xt_lag.items():
                        t_ = p1x.tile([128, KI, 128], bf16, tag=f"x{key}")
                        nc.sync.dma_start(
                            t_[:], d.rearrange("(k p) r -> p k r", p=128)[:, :, cols])
                        xs[key] = t_

                    for n in range(NCH):
                        nsl = slice(n * 512, (n + 1) * 512)
                        ps = p1ps.tile([128, 512], f32, tag="pps")
                        first = True
                        for key in ("d", "w", "m"):
                            for k in range(KI):
                                nc.tensor.matmul(ps[:], xs[key][:, k, :],
                                                 wxA_sb[key][:, k, nsl],
                                                 start=first, stop=(key == "m" and k == KI - 1))
                                first = False
                        st = p1s.tile([128, 512], bf16, tag="pst")
                        nc.scalar.copy(st[:], ps[:])
                        nc.sync.dma_start(p_all[r, 4, :, nsl], st[:])

                        ps = p1ps.tile([128, 512], f32, tag="pps")
                        for k in range(KI):
                            nc.tensor.matmul(ps[:], xs["x"][:, k, :],
                                             wxA_sb["g"][:, k, nsl],
                                             start=(k == 0), stop=(k == KI - 1))
                        st = p1s.tile([128, 512], bf16, tag="pst")
                        nc.vector.tensor_add(st[:], ps[:], bg_sb[:, nsl])
                        nc.sync.dma_start(p_all[r, 3, :, nsl], st[:])

            # --- pass B: p_i, p_f, p_o (x-part + e-part + bias) ---
            with tc.tile_pool(name="p1B", bufs=1) as p1B, \
                 tc.tile_pool(name="p1xb", bufs=4) as p1xb, \
                 tc.tile_pool(name="p1sb", bufs=6) as p1sb, \
                 tc.tile_pool(name="p1psb", bufs=6, space="PSUM") as p1psb:

                wx_sb = {}
                we_sb = {}
                bias_sb = {}
                for g in ("i", "f", "o"):
                    t_ = p1B.tile([128, KI, H], bf16, tag=f"wxb{g}")
                    nc.gpsimd.dma_start(t_[:], wxT[g].rearrange("(k p) h -> p k h", p=128))
                    wx_sb[g] = t_
                    t2_ = p1B.tile([128, KH, H], bf16, tag=f"web{g}")
                    nc.gpsimd.dma_start(t2_[:], weT[g].rearrange("(k p) h -> p k h", p=128))
                    we_sb[g] = t2_
                    t3_ = p1B.tile([128, H], f32, tag=f"bb{g}")
                    nc.gpsimd.dma_start(t3_[:], bcast(bias[g]))
                    bias_sb[g] = t3_

                for r in range(T):
                    cols = slice(r * BC, (r + 1) * BC)
                    xx = p1xb.tile([128, KI, 128], bf16, tag="xxb")
                    nc.sync.dma_start(
                        xx[:], xt_in.rearrange("(k p) r -> p k r", p=128)[:, :, cols])
                    er = p1xb.tile([128, KH, 128], bf16, tag="er")
                    nc.sync.dma_start(
                        er[:], eT_d[:, :, cols].rearrange("k p r -> p k r"))

                    for n in range(NCH):
                        nsl = slice(n * 512, (n + 1) * 512)
                        for gi, g in enumerate(("i", "f", "o")):
                            ps = p1psb.tile([128, 512], f32, tag="ppsb")
                            for k in range(KI):
                                nc.tensor.matmul(ps[:], xx[:, k, :],
                                                 wx_sb[g][:, k, nsl],
                                                 start=(k == 0), stop=False)
                            for k in range(KH):
                                nc.tensor.matmul(ps[:], er[:, k, :],
                                                 we_sb[g][:, k, nsl],
                                                 start=False, stop=(k == KH - 1))
                            st = p1sb.tile([128, 512], bf16, tag="pstb")
                            nc.vector.tensor_add(st[:], ps[:], bias_sb[g][:, nsl])
                            nc.sync.dma_start(p_all[r, gi, :, nsl], st[:])

            # ================= PHASE 2 =================
            with tc.tile_pool(name="p2s", bufs=2) as p2s, \
                 tc.tile_pool(name="p2p", bufs=3) as p2p, \
                 tc.tile_pool(name="p2t", bufs=2) as p2t, \
                 tc.tile_pool(name="p2ps", bufs=5, space="PSUM") as p2ps, \
                 tc.tile_pool(name="p2tr", bufs=2, space="PSUM") as p2tr:

                hT = None   # [128, KH*128] bf16, slice k = lhsT k-tile of h
                c_prev = None

                for t in range(T):
                    p5 = p2p.tile([128, 5, H], bf16, tag="p5")
                    nc.sync.dma_start(p5[:], p_all[t].rearrange("g p h -> p g h"))

                    # ---- h_o = sigmoid(p_per + 2 h w_t.T) ----
                    ho_bf = p2s.tile([128, H], bf16, tag="ho")
                    for n in range(NCH):
                        nsl = slice(n * 512, (n + 1) * 512)
                        if t == 0:
                            nc.scalar.activation(ho_bf[:, nsl], p5[:, 4, nsl], SIG)
                        else:
                            ps = p2ps.tile([128, 512], f32, tag="mm")
                            for k in range(KH):
                                nc.tensor.matmul(ps[:], hT[:, k * 128:(k + 1) * 128],
                                                 wr_sb["t2"][:, k, nsl],
                                                 start=(k == 0), stop=(k == KH - 1))
                            pre = p2t.tile([128, 512], f32, tag="pre")
                            nc.vector.tensor_add(pre[:], ps[:], p5[:, 4, nsl])
                            nc.scalar.activation(ho_bf[:, nsl], pre[:], SIG)

                    # transpose h_o -> hoT, half 0 first; half 1 is emitted
                    # between the first gate k-halves so PE never waits on
                    # the second sigmoid+copy chain
                    hoT = p2s.tile([128, KH * 128], bf16, tag="hoT")
                    pst0 = p2tr.tile([128, 512], bf16, tag="tr")
                    for j4 in range(4):
                        nc.tensor.transpose(
                            pst0[:, j4 * 128:(j4 + 1) * 128],
                            ho_bf[:, j4 * 128:(j4 + 1) * 128], ident[:])
                    nc.scalar.copy(hoT[:, 0:512], pst0[:])
                    pst1 = p2tr.tile([128, 512], bf16, tag="tr")
                    for j4 in range(4):
                        nc.tensor.transpose(
                            pst1[:, j4 * 128:(j4 + 1) * 128],
                            ho_bf[:, 512 + j4 * 128:512 + (j4 + 1) * 128], ident[:])
                    nc.scalar.copy(hoT[:, 512:1024], pst1[:])

                    # ---- gates + state update, pipelined per 512-chunk ----
                    # chunk n's elementwise/transposes overlap chunk n+1's MMs
                    c_new = p2s.tile([128, H], f32, tag="c")
                    if t < T - 1:
                        hT_new = p2s.tile([128, KH * 128], bf16, tag="hT")
                    else:
                        hT_new = None
                    GATES = (("f", "fo", 1, SIG), ("i", "ih", 0, SIG),
                             ("g", "gh", 3, TANH), ("o", "oh", 2, SIG))
                    for n in range(NCH):
                        nsl = slice(n * 512, (n + 1) * 512)
                        gate = {}
                        for gidx, (g, widx, pidx, fn) in enumerate(GATES):
                            ps = p2ps.tile([128, 512], f32, tag="mm")
                            for k in range(KH):
                                nc.tensor.matmul(ps[:], hoT[:, k * 128:(k + 1) * 128],
                                                 wr_sb[widx][:, k, nsl],
                                                 start=(k == 0), stop=(k == KH - 1))
                            pre = p2t.tile([128, 512], f32, tag="pre")
                            nc.vector.tensor_add(pre[:], ps[:], p5[:, pidx, nsl])
                            gt = p2t.tile([128, 512], f32, tag=f"gate{g}")
                            nc.scalar.activation(gt[:], pre[:], fn)
                            gate[g] = gt

                        ig = p2t.tile([128, 512], f32, tag="ig")
                        nc.vector.tensor_mul(ig[:], gate["i"][:], gate["g"][:])
                        if t == 0:
                            nc.vector.tensor_copy(c_new[:, nsl], ig[:])
                        else:
                            fc = p2t.tile([128, 512], f32, tag="fc")
                            nc.vector.tensor_mul(fc[:], gate["f"][:], c_prev[:, nsl])
                            nc.vector.tensor_add(c_new[:, nsl], fc[:], ig[:])
                        tanh_c = p2t.tile([128, 512], f32, tag="tc")
                        nc.scalar.activation(tanh_c[:], c_new[:, nsl], TANH)
                        h_nat = p2t.tile([128, 512], f32, tag="h")
                        nc.vector.tensor_mul(h_nat[:], gate["o"][:], tanh_c[:])

                        nc.sync.dma_start(hs_out[:, t, nsl], h_nat[:])
                        nc.sync.dma_start(cs_out[:, t, nsl], c_new[:, nsl])

                        if hT_new is not None:
                            h_bf = p2t.tile([128, 512], bf16, tag="hbf")
                            nc.vector.tensor_copy(h_bf[:], h_nat[:])
                            pst = p2tr.tile([128, 512], bf16, tag="tr")
                            for j4 in range(4):
                                nc.tensor.transpose(
                                    pst[:, j4 * 128:(j4 + 1) * 128],
                                    h_bf[:, j4 * 128:(j4 + 1) * 128], ident[:])
                            nc.scalar.copy(hT_new[:, nsl], pst[:])
                    hT = hT_new
                    c_prev = c_new

            p2w_cm.__exit__(None, None, None)

    nc.compile()
    return nc


def _prep_host(x_input, x_weather, **w):
    """Build per-core input maps (host-side shifts/transposes/casts)."""
    def shift(k):
        out = np.zeros_like(x_input)
        out[k:] = x_input[:-k]
        return out

    lags = [x_input, shift(1), shift(7), shift(30)]
    names = ["xt_in", "xt_day", "xt_week", "xt_month"]

    shared = {
        "w_eT": np.ascontiguousarray(w["w_e"].T).astype(BF16),
        "b_e_t": np.ascontiguousarray(w["b_e"].reshape(KH, 128).T).astype(np.float32),
    }
    for g, key in (("d", "w_d"), ("w", "w_w"), ("m", "w_m"), ("i", "w_ix"),
                   ("f", "w_fx"), ("o", "w_ox"), ("g", "w_gx")):
        shared[f"w{g}xT"] = np.ascontiguousarray(w[key].T).astype(BF16)
    for g, key in (("i", "w_ie"), ("f", "w_fe"), ("o", "w_oe")):
        shared[f"w{g}eT"] = np.ascontiguousarray(w[key].T).astype(BF16)
    for g in ("i", "f", "o", "g"):
        shared[f"b_{g}"] = w[f"b_{g}"].reshape(1, H).astype(np.float32)
    for g, key in (("t2", "w_t"), ("ih", "w_ih"), ("gh", "w_gh"),
                   ("fo", "w_fo"), ("oh", "w_oh")):
        m = w[key] * 2.0 if g == "t2" else w[key]
        shared[f"wr_{g}T"] = np.ascontiguousarray(m.T).astype(BF16)

    in_maps = []
    for c in range(NCORES):
        rows = slice(c * BC, (c + 1) * BC)
        m = dict(shared)
        for nm, arr in zip(names, lags):
            # [BC, T, I] -> [I, T*BC] with col r = t*BC + b
            m[nm] = np.ascontiguousarray(
                arr[rows].transpose(2, 1, 0).reshape(I, R)).astype(BF16)
        m["xwt"] = np.ascontiguousarray(
            x_weather[rows].transpose(2, 1, 0).reshape(W, R)).astype(BF16)
        in_maps.append(m)
    return in_maps


def kernel(**inputs):
    global _CACHED
    from concourse.bass_utils import run_bass_kernel_spmd

    if _CACHED is None:
        _CACHED = _build()
    nc = _CACHED

    np_inputs = {k: np.asarray(v) for k, v in inputs.items()}
    in_maps = _prep_host(**np_inputs)
    res = run_bass_kernel_spmd(nc, in_maps, core_ids=list(range(NCORES)))

    hs = np.empty((B, T, H), np.float32)
    cs = np.empty((B, T, H), np.float32)
    for c in range(NCORES):
        rows = slice(c * BC, (c + 1) * BC)
        hs[rows] = res.results[c]["hs"]
        cs[rows] = res.results[c]["cs"]
    return hs, cs
